# revision 15
# baseline (speedup 1.0000x reference)
"""Multi-head attention (B=2, S=2048, D=1024, H=16) on 8 Trainium2 cores.

Sharding: tensor-parallel over heads (4 groups of 4 heads) x data-parallel
over batch (2). Core c handles batch c//4, head group c%4. Out-projection:
each core computes fp16 partials for all 1024 out cols from its 256 ctx
dims; ReduceScatter(add) over the 4-core group hands rank r its 256-col
quarter.

v2 pipeline (all f16 activations/weights, fp32 PSUM):
  per sk/sq group j: project k(j), v(j), q(j) from x pieces, then attention
  jg=j. Proj/out-proj matmul chains are emitted as *fillers* between
  attention blocks so TensorE never idles while ScalarE runs exp (keeps the
  HAM clock-gate warm). Scores for a head-pair land in one 2-bank PSUM tile
  [128,2,512] so a single ACT exp covers both heads. Causal masking is done
  after exp by GpSimd affine_select (zeroes the upper triangle of the diag
  block) -- no mask tensor, no VectorE work. The V tiles carry a ones
  column so the PV matmul accumulates softmax denominators for free; the
  reciprocal uses the fast DVE approx (1 op) and is broadcast across
  partitions with a tiny ones-vector matmul.
"""
import os
from collections import deque

import numpy as np

import concourse.bass as bass
import concourse.mybir as mybir
import concourse.tile as tile
import bass_rust as _bass_rust
from concourse.bass_utils import run_bass_kernel_spmd

dt = mybir.dt
AF = mybir.ActivationFunctionType
ALU = mybir.AluOpType

B, S, D, H = 2, 2048, 1024, 16
DK = D // H          # 64
HL = 4               # heads per core
DL = HL * DK         # 256 local head dims
NCORE = 8
GROUPS = [[0, 1, 2, 3], [4, 5, 6, 7]]
SQG = 512            # sq group width (one PSUM bank of fp32)
NSQG = S // SQG      # 4
NSK = S // 128       # 16 sk blocks
KCH = D // 128       # 8 contraction chunks for projections
SCALE = 1.0 / float(np.sqrt(np.float32(DK)))

F16 = dt.float16
F32 = dt.float32
F32R = dt.float32r

LAST_RESULT = None   # BassKernelResults of the most recent run (profiling)
_CACHE = {}          # causal -> built Bass


def _split_multiwait(nc):
    """This walrus supports one sync-wait per instruction; Tile emits several.
    Hoist all but the last wait of each instruction onto single-wait NOPs
    placed immediately before it on the same engine."""
    for bbw in nc.bb_map.values():
        insts = bbw.bb.instructions
        out = []
        for inst in insts:
            si = inst.sync_info
            waits = list(si.on_wait or []) if si is not None else []
            if len(waits) > 1:
                for w in waits[:-1]:
                    nop = _bass_rust.InstNoOp(
                        name=nc.get_next_instruction_name(), ins=[], outs=[])
                    nop.engine = inst.engine
                    nop.bass_nofuse = True
                    nop.sync_info = mybir.SyncInfo(on_wait=[w], on_update=[])
                    nc.register_instruction(nop)
                    out.append(nop)
                inst.sync_info = mybir.SyncInfo(
                    on_wait=[waits[-1]], on_update=list(si.on_update or []))
            out.append(inst)
        insts[:] = out


def _build(causal: bool):
    nc = bass.Bass(num_devices=NCORE)

    xq = nc.declare_dram_parameter("xq", [D, S], F16, isOutput=False)
    xk = nc.declare_dram_parameter("xk", [D, S], F16, isOutput=False)
    xv = nc.declare_dram_parameter("xv", [D, S], F16, isOutput=False)
    wq = nc.declare_dram_parameter("wq", [D, DL], F16, isOutput=False)
    wk = nc.declare_dram_parameter("wk", [D, DL], F16, isOutput=False)
    wv = nc.declare_dram_parameter("wv", [D, DL], F16, isOutput=False)
    wo = nc.declare_dram_parameter("wo", [DL, D], F16, isOutput=False)
    out = nc.declare_dram_parameter("out", [2, 128, S], F16, isOutput=True)
    debug = os.environ.get("KERNEL_DEBUG", "0") == "1"
    if debug:
        d_qT = nc.declare_dram_parameter("d_qT", [2, 128, S], F16, isOutput=True)
        d_kT = nc.declare_dram_parameter("d_kT", [2, 128, S], F16, isOutput=True)
        d_Vp = nc.declare_dram_parameter("d_Vp", [128, NSK, 65 * HL], F16, isOutput=True)
        d_ctx = nc.declare_dram_parameter("d_ctx", [128, 2, S], F16, isOutput=True)

    with tile.TileContext(nc) as tc:
        with (
            tc.tile_pool(name="wpool", bufs=1) as wpool,
            tc.tile_pool(name="xpool", bufs=4) as xpool,
            tc.tile_pool(name="apool", bufs=1) as apool,
            tc.tile_pool(name="epool", bufs=3) as epool,
            tc.tile_pool(name="opool", bufs=2) as opool,
            tc.tile_pool(name="psS", bufs=2, space="PSUM") as psS,
            tc.tile_pool(name="psC", bufs=1, space="PSUM") as psC,
            tc.tile_pool(name="psP", bufs=1, space="PSUM") as psP,
            tc.tile_pool(name="dram", bufs=1, space="DRAM") as drp,
        ):
            # ---- resident weights / constants ----
            wq_sb = wpool.tile([128, KCH, DL], F16, tag="wq")
            wk_sb = wpool.tile([128, KCH, DL], F16, tag="wk")
            wv_sb = wpool.tile([128, KCH, DL], F16, tag="wv")
            wo_sb = wpool.tile([128, 2, D], F16, tag="wo")
            for wsb, wsrc in ((wq_sb, wq), (wk_sb, wk), (wv_sb, wv)):
                for c2 in range(2):  # split across queues
                    nc.sync.dma_start(
                        wsb[:, 4 * c2:4 * (c2 + 1), :],
                        wsrc.rearrange("(c p) m -> p c m", p=128)[:, 4 * c2:4 * (c2 + 1), :])
            nc.sync.dma_start(wo_sb[:], wo.rearrange("(c p) m -> p c m", p=128))
            ones64f = wpool.tile([1, 64], F32, tag="ones64f")
            nc.vector.memset(ones64f[:], 1.0)
            ones64 = wpool.tile([1, 64], F32R, tag="ones64")
            nc.vector.tensor_copy(ones64[:], ones64f[:])

            # ---- persistent activations ----
            qT = [apool.tile([128, S], F16, tag=f"qT{hp}", name=f"qT{hp}")
                  for hp in range(2)]
            kT = [apool.tile([128, S], F16, tag=f"kT{hp}", name=f"kT{hp}")
                  for hp in range(2)]
            Vp = apool.tile([128, NSK, 65 * HL], F16, tag="Vp")
            nc.gpsimd.memset(
                Vp.rearrange("p i (h e) -> p i h e", e=65)[:, :, :, 64:65], 1.0)
            ctx_sb = apool.tile([128, 2, S], F16, tag="ctx")

            # ---- x pieces: [128, KCH, SQG] per (tensor, group), all live ----
            xk_pc, xv_pc, xq_pc = [], [], []
            for j in range(NSQG):
                tk = xpool.tile([128, KCH, SQG], F16, tag="xk", name=f"xk{j}")
                tv = xpool.tile([128, KCH, SQG], F16, tag="xv", name=f"xv{j}")
                tq = xpool.tile([128, KCH, SQG], F16, tag="xq", name=f"xq{j}")
                xk_pc.append(tk)
                xv_pc.append(tv)
                xq_pc.append(tq)
            for j in range(NSQG):
                for (tl, src) in ((xk_pc[j], xk), (xv_pc[j], xv), (xq_pc[j], xq)):
                    for kk in range(KCH):
                        nc.sync.dma_start(
                            tl[:, kk, :],
                            src[128 * kk:128 * (kk + 1), SQG * j:SQG * (j + 1)])

            # ---------------- filler chain machinery ----------------
            # Each chain-unit is a list of thunks; consecutive thunks of the
            # open unit are popped between attention blocks. A unit owns one
            # psP tile for its whole life, so units must not interleave.
            fillers = deque()   # deque of lists (chain units); unit = deque of thunks

            def drain(n):
                """Emit up to n filler thunks (crossing unit boundaries)."""
                while n > 0 and fillers:
                    unit = fillers[0]
                    while n > 0 and unit:
                        unit.popleft()()
                        n -= 1
                    if not unit:
                        fillers.popleft()

            def drain_unit_boundary():
                """Finish the currently open chain unit (frees its psP tile)."""
                if fillers and fillers[0]:
                    unit = fillers.popleft()
                    while unit:
                        unit.popleft()()

            def drain_all():
                while fillers:
                    drain_unit_boundary()

            # ---------------- projection chain units ----------------
            def qk_proj_unit(j, xt, w_sb, dst):
                """One unit: both 128-row halves of q/k columns for group j."""
                unit = deque()
                state = {}

                def open_():
                    state["ps"] = psP.tile([128, 2, SQG], F32, tag="pj",
                                           name=f"pj_{id(state)}")
                for kk in range(KCH):
                    def mm(kk=kk):
                        if kk == 0:
                            open_()
                        ps = state["ps"]
                        for cc in range(2):
                            nc.tensor.matmul(
                                ps[:, cc, :],
                                lhsT=w_sb[:, kk, 128 * cc:128 * (cc + 1)],
                                rhs=xt[:, kk, :],
                                start=(kk == 0), stop=(kk == KCH - 1))
                    unit.append(mm)

                def close():
                    ps = state["ps"]
                    for cc in range(2):
                        nc.vector.tensor_copy(
                            dst[cc][:, SQG * j:SQG * (j + 1)], ps[:, cc, :])
                unit.append(close)
                return unit

            def v_proj_unit(j, half):
                """One unit: two sk-128-chunks of v for group j (natural)."""
                unit = deque()
                state = {}
                for kk in range(KCH):
                    def mm(kk=kk):
                        if kk == 0:
                            state["ps"] = psP.tile([128, 2, SQG], F32, tag="pj",
                                                   name=f"pv_{id(state)}")
                        ps = state["ps"]
                        for sc2 in range(2):
                            sc = 2 * half + sc2
                            nc.tensor.matmul(
                                ps[:, sc2, :DL],
                                lhsT=xv_pc[j][:, kk, 128 * sc:128 * (sc + 1)],
                                rhs=wv_sb[:, kk, :],
                                start=(kk == 0), stop=(kk == KCH - 1))
                    unit.append(mm)

                def close():
                    ps = state["ps"]
                    for sc2 in range(2):
                        sc = 2 * half + sc2
                        i = 4 * j + sc
                        vdst = Vp[:, i].rearrange("p (h e) -> p h e", e=65)
                        nc.vector.tensor_copy(
                            vdst[:, :, :64],
                            ps[:, sc2, :DL].rearrange("p (h e) -> p h e", e=64))
                unit.append(close)
                return unit

            def proj_units(j):
                """Chain units for group j in dependency-useful order."""
                return [
                    qk_proj_unit(j, xk_pc[j], wk_sb, kT),
                    v_proj_unit(j, 0),
                    v_proj_unit(j, 1),
                    qk_proj_unit(j, xq_pc[j], wq_sb, qT),
                ]

            # ---------------- out-projection + ReduceScatter ----------------
            def outproj_units(jg):
                """4 units x (2 oc chains of 2 MMs + copies); last unit also
                stages DRAM part + ReduceScatter + final out DMA."""
                par = {}

                def open_par():
                    par["sb"] = opool.tile([128, KCH, SQG], F16, tag="par",
                                           name=f"par{jg}")
                units = []
                for u in range(4):
                    unit = deque()
                    for oc2 in range(2):
                        oc = 2 * u + oc2
                        def mm(oc=oc, oc2=oc2, u=u):
                            if u == 0 and oc2 == 0:
                                open_par()
                            if oc2 == 0:
                                par["ps"] = psP.tile([128, 2, SQG], F32, tag="pj",
                                                     name=f"po{jg}_{u}")
                            ps = par["ps"]
                            for kc in range(2):
                                nc.tensor.matmul(
                                    ps[:, oc2, :],
                                    lhsT=wo_sb[:, kc, 128 * oc:128 * (oc + 1)],
                                    rhs=ctx_sb[:, kc, SQG * jg:SQG * (jg + 1)],
                                    start=(kc == 0), stop=(kc == 1))
                        unit.append(mm)

                    def close(u=u):
                        ps = par["ps"]
                        for oc2 in range(2):
                            nc.vector.tensor_copy(
                                par["sb"][:, 2 * u + oc2, :], ps[:, oc2, :])
                    unit.append(close)
                    units.append(unit)

                def ship():
                    part = drp.tile([KCH, 128, SQG], F16, name=f"part{jg}")
                    for oc in range(KCH):
                        nc.sync.dma_start(part[oc], par["sb"][:, oc, :])
                    rsout = drp.tile([2, 128, SQG], F16, name=f"rso{jg}")
                    nc.gpsimd.collective_compute(
                        "ReduceScatter", ALU.add, replica_groups=GROUPS,
                        ins=[part.opt()], outs=[rsout.opt()])
                    for h2 in range(2):
                        nc.sync.dma_start(
                            out[h2:h2 + 1, :, SQG * jg:SQG * (jg + 1)], rsout[h2:h2 + 1])
                units[-1].append(ship)
                return units

            # ---------------- attention ----------------
            def attn_jg(jg):
                nsk = 4 * jg + 4 if causal else NSK
                for hp in range(2):
                    ctx_ps = [psC.tile([65, SQG], F32, tag=f"ctx{m}",
                                       name=f"ctx{jg}_{hp}_{m}")
                              for m in range(2)]
                    ets = {}

                    def pv(i):
                        et, c0 = ets.pop(i)
                        for m in range(2):
                            hl = 2 * hp + m
                            nc.tensor.matmul(
                                ctx_ps[m][:, c0:SQG],
                                lhsT=Vp[:, i, 65 * hl:65 * hl + 65],
                                rhs=et[:, m, c0:SQG],
                                start=(i == 0), stop=(i == nsk - 1))

                    for i in range(nsk):
                        col0 = 128 * max(0, i - 4 * jg) if causal else 0
                        sps = psS.tile([128, 2, SQG], F32, tag="sps",
                                       name=f"sps{jg}_{hp}_{i}")
                        for m in range(2):
                            nc.tensor.matmul(
                                sps[:, m, col0:SQG],
                                lhsT=kT[hp][64 * m:64 * m + 64,
                                            128 * i:128 * (i + 1)],
                                rhs=qT[hp][64 * m:64 * m + 64,
                                           SQG * jg + col0:SQG * (jg + 1)],
                                start=True, stop=True)
                        et = epool.tile([128, 2, SQG], F16, tag="exp",
                                        name=f"exp{jg}_{hp}_{i}")
                        nc.scalar.activation(
                            et[:, :, col0:SQG], sps[:, :, col0:SQG],
                            AF.Exp, scale=SCALE)
                        if causal and i >= 4 * jg:
                            # zero strictly-upper triangle of the diagonal
                            # 128x128 sub-block: keep where (f - p) >= 0
                            nc.gpsimd.affine_select(
                                et[:, :, col0:col0 + 128],
                                et[:, :, col0:col0 + 128],
                                pattern=[[0, 2], [1, 128]],
                                compare_op=ALU.is_ge,
                                fill=0.0,
                                base=0,
                                channel_multiplier=-1)
                        ets[i] = (et, col0)
                        drain(2)
                        if i > 0:
                            pv(i - 1)
                    pv(nsk - 1)

                    # ---- softmax finalize: recip + broadcast + scale ----
                    # DVE reciprocal is an 8-cycle/elem iterative divide, so
                    # computing it on a [1,512] row costs ~3.3us. Spread the
                    # 1024 denominators over 16 partitions via tiny
                    # SBUF->SBUF DMAs, recip there at 64 elem/lane, gather
                    # back; out in f32r to satisfy the f32r-matmul rounding
                    # rule without a cast copy.
                    drain_unit_boundary()
                    den = opool.tile([1, 2 * SQG], F32, tag="den",
                                     name=f"den{jg}_{hp}")
                    for m in range(2):
                        nc.vector.tensor_copy(
                            den[:, SQG * m:SQG * (m + 1)], ctx_ps[m][64:65, :])
                    den_dr = drp.tile([16, 64], F32, name=f"dendr{jg}_{hp}")
                    nc.sync.dma_start(
                        den_dr.rearrange("p e -> (p e)").rearrange(
                            "(one f) -> one f", one=1), den[:])
                    den_sp = opool.tile([16, 64], F32, tag="densp",
                                        name=f"densp{jg}_{hp}")
                    nc.sync.dma_start(den_sp[:], den_dr[:])
                    rec_sp = opool.tile([16, 64], F32R, tag="recsp",
                                        name=f"recsp{jg}_{hp}")
                    with nc.allow_low_precision(reason="recip in f32r"):
                        nc.vector.reciprocal(rec_sp[:], den_sp[:])
                    rec_dr = drp.tile([16, 64], F32R, name=f"recdr{jg}_{hp}")
                    nc.sync.dma_start(rec_dr[:], rec_sp[:])
                    rec = opool.tile([1, 2 * SQG], F32R, tag="rec",
                                     name=f"rec{jg}_{hp}")
                    nc.sync.dma_start(
                        rec[:], rec_dr.rearrange("p e -> (p e)").rearrange(
                            "(one f) -> one f", one=1))
                    for m in range(2):
                        bc = psP.tile([128, 2, SQG], F32, tag="pj",
                                      name=f"bc{jg}_{hp}_{m}")
                        nc.tensor.matmul(bc[0:64, 0, :],
                                         lhsT=ones64[:],
                                         rhs=rec[:, SQG * m:SQG * (m + 1)],
                                         start=True, stop=True)
                        bc_sb = opool.tile([64, SQG], F32, tag=f"bcsb{m}",
                                           name=f"bcsb{jg}_{hp}_{m}")
                        nc.vector.tensor_copy(bc_sb[:], bc[0:64, 0, :])
                        nc.vector.tensor_tensor(
                            ctx_sb[64 * m:64 * m + 64, hp,
                                   SQG * jg:SQG * (jg + 1)],
                            ctx_ps[m][0:64, :],
                            bc_sb[:], ALU.mult)

            # ---------------- schedule ----------------
            for u in proj_units(0):
                while u:
                    u.popleft()()
            for jg in range(NSQG):
                # everything queued before this point produces data attn(jg)
                # may read (proj of group jg) -- it must precede attn(jg) in
                # each engine's in-order stream or the PE queue deadlocks.
                drain_all()
                if jg + 1 < NSQG:
                    fillers.extend(proj_units(jg + 1))
                if jg >= 1:
                    fillers.extend(outproj_units(jg - 1))
                attn_jg(jg)
            drain_all()
            for u in outproj_units(NSQG - 1):
                while u:
                    u.popleft()()

            if debug:
                for hp in range(2):
                    nc.sync.dma_start(d_qT[hp], qT[hp][:])
                    nc.sync.dma_start(d_kT[hp], kT[hp][:])
                    nc.sync.dma_start(d_ctx[:, hp, :], ctx_sb[:, hp, :])
                nc.sync.dma_start(d_Vp[:], Vp[:])

    _split_multiwait(nc)
    return nc


def _mask_kind(mask: np.ndarray) -> bool:
    """True if causal (tril), False if all-ones; raises otherwise."""
    m = np.asarray(mask).reshape(S, S)
    if np.array_equal((m != 0).astype(np.int8), np.tril(np.ones((S, S), np.int8))):
        return True
    if np.all(m != 0):
        return False
    raise NotImplementedError("unsupported mask pattern")


def kernel(q, k, v, mask, w_q, b_q, w_k, b_k, w_v, b_v, w_o, b_o):
    global LAST_RESULT
    assert not np.any(b_q) and not np.any(b_k) and not np.any(b_v) \
        and not np.any(b_o), "nonzero biases not supported"
    causal = _mask_kind(mask)

    if causal not in _CACHE:
        _CACHE[causal] = _build(causal)
    nc = _CACHE[causal]

    q = np.asarray(q, np.float32)
    k = np.asarray(k, np.float32)
    v = np.asarray(v, np.float32)
    # transposed per-batch activations
    xqs = [np.ascontiguousarray(q[b].T).astype(np.float16) for b in range(B)]
    xks = [np.ascontiguousarray(k[b].T).astype(np.float16) for b in range(B)]
    xvs = [np.ascontiguousarray(v[b].T).astype(np.float16) for b in range(B)]
    wqs = [np.ascontiguousarray(np.asarray(w_q, np.float32)[:, DL * g:DL * (g + 1)]).astype(np.float16) for g in range(4)]
    wks = [np.ascontiguousarray(np.asarray(w_k, np.float32)[:, DL * g:DL * (g + 1)]).astype(np.float16) for g in range(4)]
    wvs = [np.ascontiguousarray(np.asarray(w_v, np.float32)[:, DL * g:DL * (g + 1)]).astype(np.float16) for g in range(4)]
    wos = [np.ascontiguousarray(np.asarray(w_o, np.float32)[DL * g:DL * (g + 1), :]).astype(np.float16) for g in range(4)]

    in_maps = []
    for c in range(NCORE):
        b, g = c // 4, c % 4
        in_maps.append({
            "xq": xqs[b], "xk": xks[b], "xv": xvs[b],
            "wq": wqs[g], "wk": wks[g], "wv": wvs[g], "wo": wos[g],
        })
    res = run_bass_kernel_spmd(nc, in_maps, core_ids=list(range(NCORE)))
    LAST_RESULT = res

    outf = np.empty((B, S, D), np.float32)
    for c in range(NCORE):
        b, g = c // 4, c % 4
        o = res.results[c]["out"].reshape(DL, S).astype(np.float32)
        outf[b, :, DL * g:DL * (g + 1)] = o.T
    return outf


# revision 27
# speedup vs baseline: 1.1007x; 1.1007x over previous
"""Multi-head attention (B=2, S=2048, D=1024, H=16) on 8 Trainium2 cores.

Sharding: tensor-parallel over heads (4 groups of 4 heads) x data-parallel
over batch (2). Core c handles batch c//4, head group c%4. Out-projection:
each core computes fp16 partials for all 1024 out cols from its 256 ctx
dims; ReduceScatter(add) over the 4-core group hands rank r its 256-col
quarter.

v2 pipeline (all f16 activations/weights, fp32 PSUM):
  per sk/sq group j: project k(j), v(j), q(j) from x pieces, then attention
  jg=j. Proj/out-proj matmul chains are emitted as *fillers* between
  attention blocks so TensorE never idles while ScalarE runs exp (keeps the
  HAM clock-gate warm). Scores for a head-pair land in one 2-bank PSUM tile
  [128,2,512] so a single ACT exp covers both heads. Causal masking is done
  after exp by GpSimd affine_select (zeroes the upper triangle of the diag
  block) -- no mask tensor, no VectorE work. The V tiles carry a ones
  column so the PV matmul accumulates softmax denominators for free; the
  reciprocal uses the fast DVE approx (1 op) and is broadcast across
  partitions with a tiny ones-vector matmul.
"""
import os
from collections import deque

import numpy as np

import concourse.bass as bass
import concourse.mybir as mybir
import concourse.tile as tile
import bass_rust as _bass_rust
from concourse.bass_utils import run_bass_kernel_spmd

dt = mybir.dt
AF = mybir.ActivationFunctionType
ALU = mybir.AluOpType

B, S, D, H = 2, 2048, 1024, 16
DK = D // H          # 64
HL = 4               # heads per core
DL = HL * DK         # 256 local head dims
NCORE = 8
GROUPS = [[0, 1, 2, 3], [4, 5, 6, 7]]
SQG = 512            # sq group width (one PSUM bank of fp32)
NSQG = S // SQG      # 4
NSK = S // 128       # 16 sk blocks
KCH = D // 128       # 8 contraction chunks for projections
# x is staged in fp8e4m3 scaled by XS; q/k/v weights in fp8 scaled by WS.
# Projections then come out scaled by XS*WS=32; q@k scores by 32^2=1024,
# which is folded into the exp scale. v is scaled by 32, folded into w_o.
XS, WS = 1.0, 1.0
PSC = XS * WS        # 32
SCALE = 1.0 / float(np.sqrt(np.float32(DK))) / (PSC * PSC)

F16 = dt.float16
F32 = dt.float32
F32R = dt.float32r
F8 = dt.float8e4

LAST_RESULT = None   # BassKernelResults of the most recent run (profiling)
_CACHE = {}          # causal -> built Bass


def _split_multiwait(nc):
    """This walrus supports one sync-wait per instruction; Tile emits several.
    Hoist all but the last wait of each instruction onto single-wait NOPs
    placed immediately before it on the same engine."""
    for bbw in nc.bb_map.values():
        insts = bbw.bb.instructions
        out = []
        for inst in insts:
            si = inst.sync_info
            waits = list(si.on_wait or []) if si is not None else []
            if len(waits) > 1:
                for w in waits[:-1]:
                    nop = _bass_rust.InstNoOp(
                        name=nc.get_next_instruction_name(), ins=[], outs=[])
                    nop.engine = inst.engine
                    nop.bass_nofuse = True
                    nop.sync_info = mybir.SyncInfo(on_wait=[w], on_update=[])
                    nc.register_instruction(nop)
                    out.append(nop)
                inst.sync_info = mybir.SyncInfo(
                    on_wait=[waits[-1]], on_update=list(si.on_update or []))
            out.append(inst)
        insts[:] = out


def _build(causal: bool):
    nc = bass.Bass(num_devices=NCORE)

    xq = nc.declare_dram_parameter("xq", [D, S], F16, isOutput=False)
    xk = nc.declare_dram_parameter("xk", [D, S], F16, isOutput=False)
    xv = nc.declare_dram_parameter("xv", [D, S], F16, isOutput=False)
    wq = nc.declare_dram_parameter("wq", [D, DL], F16, isOutput=False)
    wk = nc.declare_dram_parameter("wk", [D, DL], F16, isOutput=False)
    wv = nc.declare_dram_parameter("wv", [D, DL], F16, isOutput=False)
    wo = nc.declare_dram_parameter("wo", [DL, D], F16, isOutput=False)
    out = nc.declare_dram_parameter("out", [2, 128, S], F16, isOutput=True)
    debug = os.environ.get("KERNEL_DEBUG", "0") == "1"
    if debug:
        d_qT = nc.declare_dram_parameter("d_qT", [2, 128, S], F16, isOutput=True)
        d_kT = nc.declare_dram_parameter("d_kT", [2, 128, S], F16, isOutput=True)
        d_Vp = nc.declare_dram_parameter("d_Vp", [128, NSK, 65 * HL], F16, isOutput=True)
        d_ctx = nc.declare_dram_parameter("d_ctx", [128, 2, S], F16, isOutput=True)

    with tile.TileContext(nc) as tc:
        with (
            tc.tile_pool(name="wpool", bufs=1) as wpool,
            tc.tile_pool(name="xpool", bufs=4) as xpool,
            tc.tile_pool(name="apool", bufs=1) as apool,
            tc.tile_pool(name="epool", bufs=3) as epool,
            tc.tile_pool(name="opool", bufs=2) as opool,
            tc.tile_pool(name="psS", bufs=2, space="PSUM") as psS,
            tc.tile_pool(name="psC", bufs=1, space="PSUM") as psC,
            tc.tile_pool(name="psP", bufs=1, space="PSUM") as psP,
            tc.tile_pool(name="dram", bufs=1, space="DRAM") as drp,
        ):
            # ---- resident weights / constants ----
            wq_sb = wpool.tile([128, KCH, DL], F16, tag="wq")
            wk_sb = wpool.tile([128, KCH, DL], F16, tag="wk")
            wv_sb = wpool.tile([128, KCH, DL], F16, tag="wv")
            wo_sb = wpool.tile([128, 2, D], F16, tag="wo")
            for wsb, wsrc in ((wq_sb, wq), (wk_sb, wk), (wv_sb, wv)):
                for c2 in range(2):  # split across queues
                    nc.scalar.dma_start(
                        wsb[:, 4 * c2:4 * (c2 + 1), :],
                        wsrc.rearrange("(c p) m -> p c m", p=128)[:, 4 * c2:4 * (c2 + 1), :])
            ones64f = wpool.tile([1, 64], F32, tag="ones64f")
            nc.vector.memset(ones64f[:], 1.0)
            ones64 = wpool.tile([1, 64], F32R, tag="ones64")
            nc.vector.tensor_copy(ones64[:], ones64f[:])

            # ---- persistent activations ----
            qT = [apool.tile([128, S], F16, tag=f"qT{hp}", name=f"qT{hp}")
                  for hp in range(2)]
            kT = [apool.tile([128, S], F16, tag=f"kT{hp}", name=f"kT{hp}")
                  for hp in range(2)]
            Vp = apool.tile([128, NSK, 65 * HL], F16, tag="Vp")
            nc.gpsimd.memset(
                Vp.rearrange("p i (h e) -> p i h e", e=65)[:, :, :, 64:65], 1.0)
            ctx_sb = apool.tile([128, 2, S], F16, tag="ctx")

            # ---- x pieces: [128, KCH, SQG] per (tensor, group), all live ----
            # Alternate the issuing engine: SP and ACT have separate HWDGE
            # queue sets, so this doubles staging bandwidth.
            xk_pc, xv_pc, xq_pc = [], [], []
            for j in range(NSQG):
                tk = xpool.tile([128, KCH, SQG], F16, tag="xk", name=f"xk{j}")
                tv = xpool.tile([128, KCH, SQG], F16, tag="xv", name=f"xv{j}")
                tq = xpool.tile([128, KCH, SQG], F16, tag="xq", name=f"xq{j}")
                xk_pc.append(tk)
                xv_pc.append(tv)
                xq_pc.append(tq)
            dma_eng = [nc.sync, nc.scalar]
            n_dma = 0
            for j in range(NSQG):
                for (tl, src) in ((xk_pc[j], xk), (xv_pc[j], xv), (xq_pc[j], xq)):
                    for kk in range(KCH):
                        dma_eng[n_dma % 2].dma_start(
                            tl[:, kk, :],
                            src[128 * kk:128 * (kk + 1), SQG * j:SQG * (j + 1)])
                        n_dma += 1
            # wo is first needed by outproj(0) during attn(1); stage it last
            nc.scalar.dma_start(wo_sb[:], wo.rearrange("(c p) m -> p c m", p=128))

            # ---------------- filler chain machinery ----------------
            # Each chain-unit is a list of thunks; consecutive thunks of the
            # open unit are popped between attention blocks. A unit owns one
            # psP tile for its whole life, so units must not interleave.
            fillers = deque()   # deque of lists (chain units); unit = deque of thunks

            def drain(n):
                """Emit up to n filler thunks (crossing unit boundaries)."""
                while n > 0 and fillers:
                    unit = fillers[0]
                    while n > 0 and unit:
                        unit.popleft()()
                        n -= 1
                    if not unit:
                        fillers.popleft()

            def drain_unit_boundary():
                """Finish the currently open chain unit (frees its psP tile)."""
                if fillers and fillers[0]:
                    unit = fillers.popleft()
                    while unit:
                        unit.popleft()()

            def drain_all():
                while fillers:
                    drain_unit_boundary()

            # ---------------- projection chain units ----------------
            def qk_proj_unit(j, xt, w_sb, dst):
                """One unit: both 128-row halves of q/k columns for group j."""
                unit = deque()
                state = {}

                def open_():
                    state["ps"] = psP.tile([128, 2, SQG], F32, tag="pj",
                                           name=f"pj_{id(state)}")
                for kk in range(KCH):
                    def mm(kk=kk):
                        if kk == 0:
                            open_()
                        ps = state["ps"]
                        for cc in range(2):
                            nc.tensor.matmul(
                                ps[:, cc, :],
                                lhsT=w_sb[:, kk, 128 * cc:128 * (cc + 1)],
                                rhs=xt[:, kk, :],
                                start=(kk == 0), stop=(kk == KCH - 1))
                    unit.append(mm)

                def close():
                    ps = state["ps"]
                    for cc in range(2):
                        nc.vector.tensor_copy(
                            dst[cc][:, SQG * j:SQG * (j + 1)], ps[:, cc, :])
                unit.append(close)
                return unit

            def v_proj_unit(j, half):
                """One unit: two sk-128-chunks of v for group j (natural)."""
                unit = deque()
                state = {}
                for kk in range(KCH):
                    def mm(kk=kk):
                        if kk == 0:
                            state["ps"] = psP.tile([128, 2, SQG], F32, tag="pj",
                                                   name=f"pv_{id(state)}")
                        ps = state["ps"]
                        for sc2 in range(2):
                            sc = 2 * half + sc2
                            nc.tensor.matmul(
                                ps[:, sc2, :DL],
                                lhsT=xv_pc[j][:, kk, 128 * sc:128 * (sc + 1)],
                                rhs=wv_sb[:, kk, :],
                                start=(kk == 0), stop=(kk == KCH - 1))
                    unit.append(mm)

                def close():
                    ps = state["ps"]
                    for sc2 in range(2):
                        sc = 2 * half + sc2
                        i = 4 * j + sc
                        vdst = Vp[:, i].rearrange("p (h e) -> p h e", e=65)
                        nc.vector.tensor_copy(
                            vdst[:, :, :64],
                            ps[:, sc2, :DL].rearrange("p (h e) -> p h e", e=64))
                unit.append(close)
                return unit

            def proj_units(j):
                """Chain units for group j in dependency-useful order."""
                return [
                    qk_proj_unit(j, xk_pc[j], wk_sb, kT),
                    v_proj_unit(j, 0),
                    v_proj_unit(j, 1),
                    qk_proj_unit(j, xq_pc[j], wq_sb, qT),
                ]

            # ---------------- out-projection + ReduceScatter ----------------
            def outproj_units(jg):
                """4 units x (2 oc chains of 2 MMs + copies); last unit also
                stages DRAM part + ReduceScatter + final out DMA."""
                par = {}

                def open_par():
                    par["sb"] = opool.tile([128, KCH, SQG], F16, tag="par",
                                           name=f"par{jg}")
                units = []
                for u in range(4):
                    unit = deque()
                    for oc2 in range(2):
                        oc = 2 * u + oc2
                        def mm(oc=oc, oc2=oc2, u=u):
                            if u == 0 and oc2 == 0:
                                open_par()
                            if oc2 == 0:
                                par["ps"] = psP.tile([128, 2, SQG], F32, tag="pj",
                                                     name=f"po{jg}_{u}")
                            ps = par["ps"]
                            for kc in range(2):
                                nc.tensor.matmul(
                                    ps[:, oc2, :],
                                    lhsT=wo_sb[:, kc, 128 * oc:128 * (oc + 1)],
                                    rhs=ctx_sb[:, kc, SQG * jg:SQG * (jg + 1)],
                                    start=(kc == 0), stop=(kc == 1))
                        unit.append(mm)

                    def close(u=u):
                        ps = par["ps"]
                        for oc2 in range(2):
                            nc.vector.tensor_copy(
                                par["sb"][:, 2 * u + oc2, :], ps[:, oc2, :])
                    unit.append(close)
                    units.append(unit)

                def ship():
                    part = drp.tile([KCH, 128, SQG], F16, name=f"part{jg}")
                    for oc in range(KCH):
                        nc.sync.dma_start(part[oc], par["sb"][:, oc, :])
                    rsout = drp.tile([2, 128, SQG], F16, name=f"rso{jg}")
                    nc.gpsimd.collective_compute(
                        "ReduceScatter", ALU.add, replica_groups=GROUPS,
                        ins=[part.opt()], outs=[rsout.opt()])
                    for h2 in range(2):
                        nc.sync.dma_start(
                            out[h2:h2 + 1, :, SQG * jg:SQG * (jg + 1)], rsout[h2:h2 + 1])
                units[-1].append(ship)
                return units

            # ---------------- attention ----------------
            def attn_jg(jg):
                nsk = 4 * jg + 4 if causal else NSK
                for hp in range(2):
                    ctx_ps = [psC.tile([65, SQG], F32, tag=f"ctx{m}",
                                       name=f"ctx{jg}_{hp}_{m}")
                              for m in range(2)]
                    ets = {}

                    def pv(i):
                        et, c0 = ets.pop(i)
                        for m in range(2):
                            hl = 2 * hp + m
                            nc.tensor.matmul(
                                ctx_ps[m][:, c0:SQG],
                                lhsT=Vp[:, i, 65 * hl:65 * hl + 65],
                                rhs=et[:, m, c0:SQG],
                                start=(i == 0), stop=(i == nsk - 1))

                    for i in range(nsk):
                        col0 = 128 * max(0, i - 4 * jg) if causal else 0
                        sps = psS.tile([128, 2, SQG], F32, tag="sps",
                                       name=f"sps{jg}_{hp}_{i}")
                        for m in range(2):
                            nc.tensor.matmul(
                                sps[:, m, col0:SQG],
                                lhsT=kT[hp][64 * m:64 * m + 64,
                                            128 * i:128 * (i + 1)],
                                rhs=qT[hp][64 * m:64 * m + 64,
                                           SQG * jg + col0:SQG * (jg + 1)],
                                start=True, stop=True)
                        et = epool.tile([128, 2, SQG], F16, tag="exp",
                                        name=f"exp{jg}_{hp}_{i}")
                        nc.scalar.activation(
                            et[:, :, col0:SQG], sps[:, :, col0:SQG],
                            AF.Exp, scale=SCALE)
                        if causal and i >= 4 * jg:
                            # zero strictly-upper triangle of the diagonal
                            # 128x128 sub-block: keep where (f - p) >= 0
                            nc.gpsimd.affine_select(
                                et[:, :, col0:col0 + 128],
                                et[:, :, col0:col0 + 128],
                                pattern=[[0, 2], [1, 128]],
                                compare_op=ALU.is_ge,
                                fill=0.0,
                                base=0,
                                channel_multiplier=-1)
                        ets[i] = (et, col0)
                        drain(2)
                        if i > 0:
                            pv(i - 1)
                    pv(nsk - 1)

                    # ---- softmax finalize ----
                    # Copy denominators AND raw ctx out of PSUM immediately:
                    # the ctx bank ring (bufs=1) gates the next (hp,jg)'s PV
                    # chain, so its last reader must come as early as
                    # possible. The recip/broadcast/scale then runs entirely
                    # from SBUF, overlapped with the next attention rows.
                    den = opool.tile([1, 2 * SQG], F32, tag="den",
                                     name=f"den{jg}_{hp}")
                    craw = opool.tile([128, SQG], F32, tag="craw",
                                      name=f"craw{jg}_{hp}")
                    for m in range(2):
                        nc.vector.tensor_copy(
                            den[:, SQG * m:SQG * (m + 1)], ctx_ps[m][64:65, :])
                        nc.vector.tensor_copy(
                            craw[64 * m:64 * m + 64, :], ctx_ps[m][0:64, :])
                    # DVE reciprocal is an 8-cycle/elem iterative divide
                    # (~3.3us on a [1,512] row). Spread the 1024 denominators
                    # over 16 partitions via a DRAM bounce, recip there at 64
                    # elem/lane, gather back; f32r out satisfies the
                    # f32r-matmul rounding rule without a cast copy.
                    den_dr = drp.tile([16, 64], F32, name=f"dendr{jg}_{hp}")
                    nc.sync.dma_start(
                        den_dr.rearrange("p e -> (p e)").rearrange(
                            "(one f) -> one f", one=1), den[:])
                    den_sp = opool.tile([16, 64], F32, tag="densp",
                                        name=f"densp{jg}_{hp}")
                    nc.sync.dma_start(den_sp[:], den_dr[:])
                    rec_sp = opool.tile([16, 64], F32R, tag="recsp",
                                        name=f"recsp{jg}_{hp}")
                    with nc.allow_low_precision(reason="recip in f32r"):
                        nc.vector.reciprocal(rec_sp[:], den_sp[:])
                    rec_dr = drp.tile([16, 64], F32R, name=f"recdr{jg}_{hp}")
                    nc.sync.dma_start(rec_dr[:], rec_sp[:])
                    rec = opool.tile([1, 2 * SQG], F32R, tag="rec",
                                     name=f"rec{jg}_{hp}")
                    nc.sync.dma_start(
                        rec[:], rec_dr.rearrange("p e -> (p e)").rearrange(
                            "(one f) -> one f", one=1))
                    drain_unit_boundary()
                    for m in range(2):
                        bc = psP.tile([128, 2, SQG], F32, tag="pj",
                                      name=f"bc{jg}_{hp}_{m}")
                        nc.tensor.matmul(bc[0:64, 0, :],
                                         lhsT=ones64[:],
                                         rhs=rec[:, SQG * m:SQG * (m + 1)],
                                         start=True, stop=True)
                        bc_sb = opool.tile([128, SQG], F32, tag="bcsb",
                                           name=f"bcsb{jg}_{hp}_{m}")
                        nc.vector.tensor_copy(
                            bc_sb[64 * m:64 * m + 64, :], bc[0:64, 0, :])
                        nc.vector.tensor_tensor(
                            ctx_sb[64 * m:64 * m + 64, hp,
                                   SQG * jg:SQG * (jg + 1)],
                            craw[64 * m:64 * m + 64, :],
                            bc_sb[64 * m:64 * m + 64, :], ALU.mult)

            # ---------------- schedule ----------------
            for u in proj_units(0):
                while u:
                    u.popleft()()
            for jg in range(NSQG):
                # everything queued before this point produces data attn(jg)
                # may read (proj of group jg) -- it must precede attn(jg) in
                # each engine's in-order stream or the PE queue deadlocks.
                drain_all()
                if jg + 1 < NSQG:
                    fillers.extend(proj_units(jg + 1))
                if jg >= 1:
                    fillers.extend(outproj_units(jg - 1))
                attn_jg(jg)
            drain_all()
            for u in outproj_units(NSQG - 1):
                while u:
                    u.popleft()()

            if debug:
                for hp in range(2):
                    nc.sync.dma_start(d_qT[hp], qT[hp][:])
                    nc.sync.dma_start(d_kT[hp], kT[hp][:])
                    nc.sync.dma_start(d_ctx[:, hp, :], ctx_sb[:, hp, :])
                nc.sync.dma_start(d_Vp[:], Vp[:])

    _split_multiwait(nc)
    return nc


def _mask_kind(mask: np.ndarray) -> bool:
    """True if causal (tril), False if all-ones; raises otherwise."""
    m = np.asarray(mask).reshape(S, S)
    if np.array_equal((m != 0).astype(np.int8), np.tril(np.ones((S, S), np.int8))):
        return True
    if np.all(m != 0):
        return False
    raise NotImplementedError("unsupported mask pattern")


def kernel(q, k, v, mask, w_q, b_q, w_k, b_k, w_v, b_v, w_o, b_o):
    global LAST_RESULT
    assert not np.any(b_q) and not np.any(b_k) and not np.any(b_v) \
        and not np.any(b_o), "nonzero biases not supported"
    causal = _mask_kind(mask)

    if causal not in _CACHE:
        _CACHE[causal] = _build(causal)
    nc = _CACHE[causal]

    f8 = np.float16
    q = np.asarray(q, np.float32) * XS
    k = np.asarray(k, np.float32) * XS
    v = np.asarray(v, np.float32) * XS
    # transposed per-batch activations, fp8 scaled by XS
    xqs = [np.ascontiguousarray(q[b].T).astype(f8) for b in range(B)]
    xks = [np.ascontiguousarray(k[b].T).astype(f8) for b in range(B)]
    xvs = [np.ascontiguousarray(v[b].T).astype(f8) for b in range(B)]
    # q/k/v weights fp8 scaled by WS; w_o folds away the v-path's XS*WS
    wqf = np.asarray(w_q, np.float32) * WS
    wkf = np.asarray(w_k, np.float32) * WS
    wvf = np.asarray(w_v, np.float32) * WS
    wof = np.asarray(w_o, np.float32) / PSC
    wqs = [np.ascontiguousarray(wqf[:, DL * g:DL * (g + 1)]).astype(f8) for g in range(4)]
    wks = [np.ascontiguousarray(wkf[:, DL * g:DL * (g + 1)]).astype(f8) for g in range(4)]
    wvs = [np.ascontiguousarray(wvf[:, DL * g:DL * (g + 1)]).astype(f8) for g in range(4)]
    wos = [np.ascontiguousarray(wof[DL * g:DL * (g + 1), :]).astype(np.float16) for g in range(4)]

    in_maps = []
    for c in range(NCORE):
        b, g = c // 4, c % 4
        in_maps.append({
            "xq": xqs[b], "xk": xks[b], "xv": xvs[b],
            "wq": wqs[g], "wk": wks[g], "wv": wvs[g], "wo": wos[g],
        })
    res = run_bass_kernel_spmd(nc, in_maps, core_ids=list(range(NCORE)))
    LAST_RESULT = res

    outf = np.empty((B, S, D), np.float32)
    for c in range(NCORE):
        b, g = c // 4, c % 4
        o = res.results[c]["out"].reshape(DL, S).astype(np.float32)
        outf[b, :, DL * g:DL * (g + 1)] = o.T
    return outf


# revision 34
# speedup vs baseline: 1.1646x; 1.0580x over previous
"""Multi-head attention (B=2, S=2048, D=1024, H=16) on 8 Trainium2 cores.

Sharding: tensor-parallel over heads (4 groups of 4 heads) x data-parallel
over batch (2). Core c handles batch c//4, head group c%4. Out-projection:
each core computes fp16 partials for all 1024 out cols from its 256 ctx
dims; ReduceScatter(add) over the 4-core group hands rank r its 256-col
quarter.

v2 pipeline (all f16 activations/weights, fp32 PSUM):
  per sk/sq group j: project k(j), v(j), q(j) from x pieces, then attention
  jg=j. Proj/out-proj matmul chains are emitted as *fillers* between
  attention blocks so TensorE never idles while ScalarE runs exp (keeps the
  HAM clock-gate warm). Scores for a head-pair land in one 2-bank PSUM tile
  [128,2,512] so a single ACT exp covers both heads. Causal masking is done
  after exp by GpSimd affine_select (zeroes the upper triangle of the diag
  block) -- no mask tensor, no VectorE work. The V tiles carry a ones
  column so the PV matmul accumulates softmax denominators for free; the
  reciprocal uses the fast DVE approx (1 op) and is broadcast across
  partitions with a tiny ones-vector matmul.
"""
import os
from collections import deque

import numpy as np

import concourse.bass as bass
import concourse.mybir as mybir
import concourse.tile as tile
import bass_rust as _bass_rust
from concourse.bass_utils import run_bass_kernel_spmd

dt = mybir.dt
AF = mybir.ActivationFunctionType
ALU = mybir.AluOpType

B, S, D, H = 2, 2048, 1024, 16
DK = D // H          # 64
HL = 4               # heads per core
DL = HL * DK         # 256 local head dims
NCORE = 8
GROUPS = [[0, 1, 2, 3], [4, 5, 6, 7]]
SQG = 512            # sq group width (one PSUM bank of fp32)
NSQG = S // SQG      # 4
NSK = S // 128       # 16 sk blocks
KCH = D // 128       # 8 contraction chunks for projections
# x is staged in fp8e4m3 scaled by XS; q/k/v weights in fp8 scaled by WS.
# Projections then come out scaled by XS*WS=32; q@k scores by 32^2=1024,
# which is folded into the exp scale. v is scaled by 32, folded into w_o.
XS, WS = 1.0, 1.0
PSC = XS * WS        # 32
SCALE = 1.0 / float(np.sqrt(np.float32(DK))) / (PSC * PSC)

F16 = dt.float16
F32 = dt.float32
F32R = dt.float32r
F8 = dt.float8e4

LAST_RESULT = None   # BassKernelResults of the most recent run (profiling)
_CACHE = {}          # causal -> built Bass


def _split_multiwait(nc):
    """This walrus supports one sync-wait per instruction; Tile emits several.
    Hoist all but the last wait of each instruction onto single-wait NOPs
    placed immediately before it on the same engine."""
    for bbw in nc.bb_map.values():
        insts = bbw.bb.instructions
        out = []
        for inst in insts:
            si = inst.sync_info
            waits = list(si.on_wait or []) if si is not None else []
            if len(waits) > 1:
                for w in waits[:-1]:
                    nop = _bass_rust.InstNoOp(
                        name=nc.get_next_instruction_name(), ins=[], outs=[])
                    nop.engine = inst.engine
                    nop.bass_nofuse = True
                    nop.sync_info = mybir.SyncInfo(on_wait=[w], on_update=[])
                    nc.register_instruction(nop)
                    out.append(nop)
                inst.sync_info = mybir.SyncInfo(
                    on_wait=[waits[-1]], on_update=list(si.on_update or []))
            out.append(inst)
        insts[:] = out


def _build(causal: bool):
    nc = bass.Bass(num_devices=NCORE)

    xq = nc.declare_dram_parameter("xq", [D, S], F16, isOutput=False)
    xk = nc.declare_dram_parameter("xk", [D, S], F16, isOutput=False)
    xv = nc.declare_dram_parameter("xv", [D, S], F16, isOutput=False)
    wq = nc.declare_dram_parameter("wq", [D, DL], F16, isOutput=False)
    wk = nc.declare_dram_parameter("wk", [D, DL], F16, isOutput=False)
    wv = nc.declare_dram_parameter("wv", [D, DL], F16, isOutput=False)
    wo = nc.declare_dram_parameter("wo", [DL, D], F16, isOutput=False)
    out = nc.declare_dram_parameter("out", [2, 128, S], F16, isOutput=True)
    debug = os.environ.get("KERNEL_DEBUG", "0") == "1"
    if debug:
        d_qT = nc.declare_dram_parameter("d_qT", [2, 128, S], F16, isOutput=True)
        d_kT = nc.declare_dram_parameter("d_kT", [2, 128, S], F16, isOutput=True)
        d_Vp = nc.declare_dram_parameter("d_Vp", [128, NSK, 65 * HL], F16, isOutput=True)
        d_ctx = nc.declare_dram_parameter("d_ctx", [128, 2, S], F16, isOutput=True)

    with tile.TileContext(nc) as tc:
        with (
            tc.tile_pool(name="wpool", bufs=1) as wpool,
            tc.tile_pool(name="xpool", bufs=4) as xpool,
            tc.tile_pool(name="apool", bufs=1) as apool,
            tc.tile_pool(name="epool", bufs=3) as epool,
            tc.tile_pool(name="opool", bufs=2) as opool,
            tc.tile_pool(name="psS", bufs=2, space="PSUM") as psS,
            tc.tile_pool(name="psC", bufs=1, space="PSUM") as psC,
            tc.tile_pool(name="psP", bufs=1, space="PSUM") as psP,
            tc.tile_pool(name="dram", bufs=1, space="DRAM") as drp,
        ):
            # ---- resident weights / constants ----
            wq_sb = wpool.tile([128, KCH, DL], F16, tag="wq")
            wk_sb = wpool.tile([128, KCH, DL], F16, tag="wk")
            wv_sb = wpool.tile([128, KCH, DL], F16, tag="wv")
            wo_sb = wpool.tile([128, 2, D], F16, tag="wo")
            for wsb, wsrc in ((wq_sb, wq), (wk_sb, wk), (wv_sb, wv)):
                for c2 in range(2):  # split across queues
                    nc.scalar.dma_start(
                        wsb[:, 4 * c2:4 * (c2 + 1), :],
                        wsrc.rearrange("(c p) m -> p c m", p=128)[:, 4 * c2:4 * (c2 + 1), :])
            ones64f = wpool.tile([1, 64], F32, tag="ones64f")
            nc.vector.memset(ones64f[:], 1.0)
            ones64 = wpool.tile([1, 64], F32R, tag="ones64")
            nc.vector.tensor_copy(ones64[:], ones64f[:])
            # 0/1 strictly-lower-triangular (keep f >= p) mask, doubled along
            # the head axis; built once on gpsimd while its queue is empty
            mask01 = wpool.tile([128, 2, 128], F16, tag="mask01")
            nc.vector.memset(mask01[:], 1.0)
            nc.gpsimd.affine_select(
                mask01[:], mask01[:], pattern=[[0, 2], [1, 128]],
                compare_op=ALU.is_ge, fill=0.0, base=0,
                channel_multiplier=-1)

            # ---- persistent activations ----
            qT = [apool.tile([128, S], F16, tag=f"qT{hp}", name=f"qT{hp}")
                  for hp in range(2)]
            kT = [apool.tile([128, S], F16, tag=f"kT{hp}", name=f"kT{hp}")
                  for hp in range(2)]
            Vp = apool.tile([128, NSK, 65 * HL], F16, tag="Vp")
            nc.gpsimd.memset(
                Vp.rearrange("p i (h e) -> p i h e", e=65)[:, :, :, 64:65], 1.0)
            ctx_sb = apool.tile([128, 2, S], F16, tag="ctx")

            # ---- x resident: [128, KCH, S] per tensor ----
            # Full 4KB DRAM rows per descriptor (a 512-col piece would cut
            # per-descriptor bytes 4x and leave staging descriptor-rate
            # bound). Alternate the issuing engine: SP and ACT have separate
            # HWDGE queue sets.
            xk_sb = xpool.tile([128, KCH, S], F16, tag="xk", bufs=1)
            xv_sb = xpool.tile([128, KCH, S], F16, tag="xv", bufs=1)
            xq_sb = xpool.tile([128, KCH, S], F16, tag="xq", bufs=1)
            dma_eng = [nc.sync, nc.scalar]
            for (tl, src) in ((xk_sb, xk), (xv_sb, xv), (xq_sb, xq)):
                for kk in range(KCH):
                    dma_eng[kk % 2].dma_start(
                        tl[:, kk, :], src[128 * kk:128 * (kk + 1), :])
            # wo is first needed by outproj(0) during attn(1); stage it last
            nc.scalar.dma_start(wo_sb[:], wo.rearrange("(c p) m -> p c m", p=128))
            xk_pc = [xk_sb[:, :, SQG * j:SQG * (j + 1)] for j in range(NSQG)]
            xv_pc = [xv_sb[:, :, SQG * j:SQG * (j + 1)] for j in range(NSQG)]
            xq_pc = [xq_sb[:, :, SQG * j:SQG * (j + 1)] for j in range(NSQG)]

            deferred_rs = []   # collective triggers, issued at attn boundaries

            # ---------------- filler chain machinery ----------------
            # Each chain-unit is a list of thunks; consecutive thunks of the
            # open unit are popped between attention blocks. A unit owns one
            # psP tile for its whole life, so units must not interleave.
            fillers = deque()   # deque of lists (chain units); unit = deque of thunks

            def drain(n):
                """Emit up to n filler thunks (crossing unit boundaries)."""
                while n > 0 and fillers:
                    unit = fillers[0]
                    while n > 0 and unit:
                        unit.popleft()()
                        n -= 1
                    if not unit:
                        fillers.popleft()

            def drain_unit_boundary():
                """Finish the currently open chain unit (frees its psP tile)."""
                if fillers and fillers[0]:
                    unit = fillers.popleft()
                    while unit:
                        unit.popleft()()

            def drain_all():
                while fillers:
                    drain_unit_boundary()

            # ---------------- projection chain units ----------------
            def qk_proj_unit(j, xt, w_sb, dst):
                """One unit: both 128-row halves of q/k columns for group j."""
                unit = deque()
                state = {}

                def open_():
                    state["ps"] = psP.tile([128, 2, SQG], F32, tag="pj",
                                           name=f"pj_{id(state)}")
                for kk in range(KCH):
                    def mm(kk=kk):
                        if kk == 0:
                            open_()
                        ps = state["ps"]
                        for cc in range(2):
                            nc.tensor.matmul(
                                ps[:, cc, :],
                                lhsT=w_sb[:, kk, 128 * cc:128 * (cc + 1)],
                                rhs=xt[:, kk, :],
                                start=(kk == 0), stop=(kk == KCH - 1))
                    unit.append(mm)

                def close():
                    ps = state["ps"]
                    for cc in range(2):
                        nc.vector.tensor_copy(
                            dst[cc][:, SQG * j:SQG * (j + 1)], ps[:, cc, :])
                unit.append(close)
                return unit

            def v_proj_unit(j, half):
                """One unit: two sk-128-chunks of v for group j (natural)."""
                unit = deque()
                state = {}
                for kk in range(KCH):
                    def mm(kk=kk):
                        if kk == 0:
                            state["ps"] = psP.tile([128, 2, SQG], F32, tag="pj",
                                                   name=f"pv_{id(state)}")
                        ps = state["ps"]
                        for sc2 in range(2):
                            sc = 2 * half + sc2
                            nc.tensor.matmul(
                                ps[:, sc2, :DL],
                                lhsT=xv_pc[j][:, kk, 128 * sc:128 * (sc + 1)],
                                rhs=wv_sb[:, kk, :],
                                start=(kk == 0), stop=(kk == KCH - 1))
                    unit.append(mm)

                def close():
                    ps = state["ps"]
                    for sc2 in range(2):
                        sc = 2 * half + sc2
                        i = 4 * j + sc
                        vdst = Vp[:, i].rearrange("p (h e) -> p h e", e=65)
                        nc.vector.tensor_copy(
                            vdst[:, :, :64],
                            ps[:, sc2, :DL].rearrange("p (h e) -> p h e", e=64))
                unit.append(close)
                return unit

            def proj_units(j):
                """Chain units for group j in dependency-useful order."""
                return [
                    qk_proj_unit(j, xk_pc[j], wk_sb, kT),
                    v_proj_unit(j, 0),
                    v_proj_unit(j, 1),
                    qk_proj_unit(j, xq_pc[j], wq_sb, qT),
                ]

            # ---------------- out-projection + ReduceScatter ----------------
            def outproj_units(jg):
                """4 units x (2 oc chains of 2 MMs + copies); last unit also
                stages DRAM part + ReduceScatter + final out DMA."""
                par = {}

                def open_par():
                    par["sb"] = opool.tile([128, KCH, SQG], F16, tag="par",
                                           name=f"par{jg}")
                units = []
                for u in range(4):
                    unit = deque()
                    for oc2 in range(2):
                        oc = 2 * u + oc2
                        def mm(oc=oc, oc2=oc2, u=u):
                            if u == 0 and oc2 == 0:
                                open_par()
                            if oc2 == 0:
                                par["ps"] = psP.tile([128, 2, SQG], F32, tag="pj",
                                                     name=f"po{jg}_{u}")
                            ps = par["ps"]
                            for kc in range(2):
                                nc.tensor.matmul(
                                    ps[:, oc2, :],
                                    lhsT=wo_sb[:, kc, 128 * oc:128 * (oc + 1)],
                                    rhs=ctx_sb[:, kc, SQG * jg:SQG * (jg + 1)],
                                    start=(kc == 0), stop=(kc == 1))
                        unit.append(mm)

                    def close(u=u):
                        ps = par["ps"]
                        for oc2 in range(2):
                            nc.vector.tensor_copy(
                                par["sb"][:, 2 * u + oc2, :], ps[:, oc2, :])
                    unit.append(close)
                    units.append(unit)

                def ship():
                    part = drp.tile([KCH, 128, SQG], F16, name=f"part{jg}")
                    for oc in range(KCH):
                        # split across the SP and ACT HWDGE queue sets
                        dma_eng[oc % 2].dma_start(part[oc], par["sb"][:, oc, :])

                    def fire_rs():
                        rsout = drp.tile([2, 128, SQG], F16, name=f"rso{jg}")
                        nc.gpsimd.collective_compute(
                            "ReduceScatter", ALU.add, replica_groups=GROUPS,
                            ins=[part.opt()], outs=[rsout.opt()])
                        for h2 in range(2):
                            nc.sync.dma_start(
                                out[h2:h2 + 1, :, SQG * jg:SQG * (jg + 1)],
                                rsout[h2:h2 + 1])
                    # the collective trigger WAITS on its input semaphores on
                    # the gpsimd queue; defer it so it is issued only once the
                    # par DMAs have had time to land
                    deferred_rs.append(fire_rs)
                units[-1].append(ship)
                return units

            # ---------------- attention ----------------
            def attn_jg(jg):
                nsk = 4 * jg + 4 if causal else NSK
                for hp in range(2):
                    ctx_ps = [psC.tile([65, SQG], F32, tag=f"ctx{m}",
                                       name=f"ctx{jg}_{hp}_{m}")
                              for m in range(2)]
                    ets = {}

                    def pv(i):
                        et, c0 = ets.pop(i)
                        for m in range(2):
                            hl = 2 * hp + m
                            nc.tensor.matmul(
                                ctx_ps[m][:, c0:SQG],
                                lhsT=Vp[:, i, 65 * hl:65 * hl + 65],
                                rhs=et[:, m, c0:SQG],
                                start=(i == 0), stop=(i == nsk - 1))

                    for i in range(nsk):
                        col0 = 128 * max(0, i - 4 * jg) if causal else 0
                        sps = psS.tile([128, 2, SQG], F32, tag="sps",
                                       name=f"sps{jg}_{hp}_{i}")
                        for m in range(2):
                            nc.tensor.matmul(
                                sps[:, m, col0:SQG],
                                lhsT=kT[hp][64 * m:64 * m + 64,
                                            128 * i:128 * (i + 1)],
                                rhs=qT[hp][64 * m:64 * m + 64,
                                           SQG * jg + col0:SQG * (jg + 1)],
                                start=True, stop=True)
                        et = epool.tile([128, 2, SQG], F16, tag="exp",
                                        name=f"exp{jg}_{hp}_{i}")
                        nc.scalar.activation(
                            et[:, :, col0:SQG], sps[:, :, col0:SQG],
                            AF.Exp, scale=SCALE)
                        if causal and i >= 4 * jg:
                            # zero strictly-upper triangle of the diagonal
                            # 128x128 sub-block via the 0/1 tril mask. On
                            # DVE, NOT gpsimd: collective triggers block the
                            # gpsimd queue and would stall these (and with
                            # them the PV chain).
                            nc.vector.tensor_tensor(
                                et[:, :, col0:col0 + 128],
                                et[:, :, col0:col0 + 128],
                                mask01[:], ALU.mult)
                        ets[i] = (et, col0)
                        drain(2)
                        if i > 0:
                            pv(i - 1)
                    pv(nsk - 1)

                    # ---- softmax finalize ----
                    # Copy denominators AND raw ctx out of PSUM immediately:
                    # the ctx bank ring (bufs=1) gates the next (hp,jg)'s PV
                    # chain, so its last reader must come as early as
                    # possible. The recip/broadcast/scale then runs entirely
                    # from SBUF, overlapped with the next attention rows.
                    den = opool.tile([1, 2 * SQG], F32, tag="den",
                                     name=f"den{jg}_{hp}")
                    craw = opool.tile([128, SQG], F32, tag="craw",
                                      name=f"craw{jg}_{hp}")
                    for m in range(2):
                        nc.vector.tensor_copy(
                            den[:, SQG * m:SQG * (m + 1)], ctx_ps[m][64:65, :])
                        nc.vector.tensor_copy(
                            craw[64 * m:64 * m + 64, :], ctx_ps[m][0:64, :])
                    # DVE reciprocal is an 8-cycle/elem iterative divide
                    # (~3.3us on a [1,512] row). Spread the 1024 denominators
                    # over 16 partitions via a DRAM bounce, recip there at 64
                    # elem/lane, gather back; f32r out satisfies the
                    # f32r-matmul rounding rule without a cast copy.
                    den_dr = drp.tile([16, 64], F32, name=f"dendr{jg}_{hp}")
                    nc.sync.dma_start(
                        den_dr.rearrange("p e -> (p e)").rearrange(
                            "(one f) -> one f", one=1), den[:])
                    den_sp = opool.tile([16, 64], F32, tag="densp",
                                        name=f"densp{jg}_{hp}")
                    nc.sync.dma_start(den_sp[:], den_dr[:])
                    rec_sp = opool.tile([16, 64], F32R, tag="recsp",
                                        name=f"recsp{jg}_{hp}")
                    with nc.allow_low_precision(reason="recip in f32r"):
                        nc.vector.reciprocal(rec_sp[:], den_sp[:])
                    rec_dr = drp.tile([16, 64], F32R, name=f"recdr{jg}_{hp}")
                    nc.sync.dma_start(rec_dr[:], rec_sp[:])
                    rec = opool.tile([1, 2 * SQG], F32R, tag="rec",
                                     name=f"rec{jg}_{hp}")
                    nc.sync.dma_start(
                        rec[:], rec_dr.rearrange("p e -> (p e)").rearrange(
                            "(one f) -> one f", one=1))
                    drain_unit_boundary()
                    for m in range(2):
                        bc = psP.tile([128, 2, SQG], F32, tag="pj",
                                      name=f"bc{jg}_{hp}_{m}")
                        nc.tensor.matmul(bc[0:64, 0, :],
                                         lhsT=ones64[:],
                                         rhs=rec[:, SQG * m:SQG * (m + 1)],
                                         start=True, stop=True)
                        bc_sb = opool.tile([128, SQG], F32, tag="bcsb",
                                           name=f"bcsb{jg}_{hp}_{m}")
                        nc.vector.tensor_copy(
                            bc_sb[64 * m:64 * m + 64, :], bc[0:64, 0, :])
                        nc.vector.tensor_tensor(
                            ctx_sb[64 * m:64 * m + 64, hp,
                                   SQG * jg:SQG * (jg + 1)],
                            craw[64 * m:64 * m + 64, :],
                            bc_sb[64 * m:64 * m + 64, :], ALU.mult)

            # ---------------- schedule ----------------
            for u in proj_units(0):
                while u:
                    u.popleft()()
            for jg in range(NSQG):
                # everything queued before this point produces data attn(jg)
                # may read (proj of group jg) -- it must precede attn(jg) in
                # each engine's in-order stream or the PE queue deadlocks.
                drain_all()
                if jg + 1 < NSQG:
                    fillers.extend(proj_units(jg + 1))
                if jg >= 1:
                    fillers.extend(outproj_units(jg - 1))
                attn_jg(jg)
                while deferred_rs:
                    deferred_rs.pop(0)()
            drain_all()
            for u in outproj_units(NSQG - 1):
                while u:
                    u.popleft()()
            while deferred_rs:
                deferred_rs.pop(0)()

            if debug:
                for hp in range(2):
                    nc.sync.dma_start(d_qT[hp], qT[hp][:])
                    nc.sync.dma_start(d_kT[hp], kT[hp][:])
                    nc.sync.dma_start(d_ctx[:, hp, :], ctx_sb[:, hp, :])
                nc.sync.dma_start(d_Vp[:], Vp[:])

    _split_multiwait(nc)
    return nc


def _mask_kind(mask: np.ndarray) -> bool:
    """True if causal (tril), False if all-ones; raises otherwise."""
    m = np.asarray(mask).reshape(S, S)
    if np.array_equal((m != 0).astype(np.int8), np.tril(np.ones((S, S), np.int8))):
        return True
    if np.all(m != 0):
        return False
    raise NotImplementedError("unsupported mask pattern")


def kernel(q, k, v, mask, w_q, b_q, w_k, b_k, w_v, b_v, w_o, b_o):
    global LAST_RESULT
    assert not np.any(b_q) and not np.any(b_k) and not np.any(b_v) \
        and not np.any(b_o), "nonzero biases not supported"
    causal = _mask_kind(mask)

    if causal not in _CACHE:
        _CACHE[causal] = _build(causal)
    nc = _CACHE[causal]

    f8 = np.float16
    q = np.asarray(q, np.float32) * XS
    k = np.asarray(k, np.float32) * XS
    v = np.asarray(v, np.float32) * XS
    # transposed per-batch activations, fp8 scaled by XS
    xqs = [np.ascontiguousarray(q[b].T).astype(f8) for b in range(B)]
    xks = [np.ascontiguousarray(k[b].T).astype(f8) for b in range(B)]
    xvs = [np.ascontiguousarray(v[b].T).astype(f8) for b in range(B)]
    # q/k/v weights fp8 scaled by WS; w_o folds away the v-path's XS*WS
    wqf = np.asarray(w_q, np.float32) * WS
    wkf = np.asarray(w_k, np.float32) * WS
    wvf = np.asarray(w_v, np.float32) * WS
    wof = np.asarray(w_o, np.float32) / PSC
    wqs = [np.ascontiguousarray(wqf[:, DL * g:DL * (g + 1)]).astype(f8) for g in range(4)]
    wks = [np.ascontiguousarray(wkf[:, DL * g:DL * (g + 1)]).astype(f8) for g in range(4)]
    wvs = [np.ascontiguousarray(wvf[:, DL * g:DL * (g + 1)]).astype(f8) for g in range(4)]
    wos = [np.ascontiguousarray(wof[DL * g:DL * (g + 1), :]).astype(np.float16) for g in range(4)]

    in_maps = []
    for c in range(NCORE):
        b, g = c // 4, c % 4
        in_maps.append({
            "xq": xqs[b], "xk": xks[b], "xv": xvs[b],
            "wq": wqs[g], "wk": wks[g], "wv": wvs[g], "wo": wos[g],
        })
    res = run_bass_kernel_spmd(nc, in_maps, core_ids=list(range(NCORE)))
    LAST_RESULT = res

    outf = np.empty((B, S, D), np.float32)
    for c in range(NCORE):
        b, g = c // 4, c % 4
        o = res.results[c]["out"].reshape(DL, S).astype(np.float32)
        outf[b, :, DL * g:DL * (g + 1)] = o.T
    return outf


# revision 37
# speedup vs baseline: 1.2685x; 1.0893x over previous
"""Multi-head attention (B=2, S=2048, D=1024, H=16) on 8 Trainium2 cores.

Sharding: tensor-parallel over heads (4 groups of 4 heads) x data-parallel
over batch (2). Core c handles batch c//4, head group c%4. Out-projection:
each core computes fp16 partials for all 1024 out cols from its 256 ctx
dims; ReduceScatter(add) over the 4-core group hands rank r its 256-col
quarter.

v2 pipeline (all f16 activations/weights, fp32 PSUM):
  per sk/sq group j: project k(j), v(j), q(j) from x pieces, then attention
  jg=j. Proj/out-proj matmul chains are emitted as *fillers* between
  attention blocks so TensorE never idles while ScalarE runs exp (keeps the
  HAM clock-gate warm). Scores for a head-pair land in one 2-bank PSUM tile
  [128,2,512] so a single ACT exp covers both heads. Causal masking is done
  after exp by GpSimd affine_select (zeroes the upper triangle of the diag
  block) -- no mask tensor, no VectorE work. The V tiles carry a ones
  column so the PV matmul accumulates softmax denominators for free; the
  reciprocal uses the fast DVE approx (1 op) and is broadcast across
  partitions with a tiny ones-vector matmul.
"""
import os
from collections import deque

import numpy as np

import concourse.bass as bass
import concourse.mybir as mybir
import concourse.tile as tile
import bass_rust as _bass_rust
from concourse.bass_utils import run_bass_kernel_spmd

dt = mybir.dt
AF = mybir.ActivationFunctionType
ALU = mybir.AluOpType

B, S, D, H = 2, 2048, 1024, 16
DK = D // H          # 64
HL = 4               # heads per core
DL = HL * DK         # 256 local head dims
NCORE = 8
GROUPS = [[0, 1, 2, 3], [4, 5, 6, 7]]
SQG = 512            # sq group width (one PSUM bank of fp32)
NSQG = S // SQG      # 4
NSK = S // 128       # 16 sk blocks
KCH = D // 128       # 8 contraction chunks for projections
# x is staged in fp8e4m3 scaled by XS; q/k/v weights in fp8 scaled by WS.
# Projections then come out scaled by XS*WS=32; q@k scores by 32^2=1024,
# which is folded into the exp scale. v is scaled by 32, folded into w_o.
XS, WS = 1.0, 1.0
PSC = XS * WS        # 32
SCALE = 1.0 / float(np.sqrt(np.float32(DK))) / (PSC * PSC)

F16 = dt.float16
F32 = dt.float32
F32R = dt.float32r
F8 = dt.float8e4

LAST_RESULT = None   # BassKernelResults of the most recent run (profiling)
_CACHE = {}          # causal -> built Bass


def _split_multiwait(nc):
    """This walrus supports one sync-wait per instruction; Tile emits several.
    Hoist all but the last wait of each instruction onto single-wait NOPs
    placed immediately before it on the same engine."""
    for bbw in nc.bb_map.values():
        insts = bbw.bb.instructions
        out = []
        for inst in insts:
            si = inst.sync_info
            waits = list(si.on_wait or []) if si is not None else []
            if len(waits) > 1:
                for w in waits[:-1]:
                    nop = _bass_rust.InstNoOp(
                        name=nc.get_next_instruction_name(), ins=[], outs=[])
                    nop.engine = inst.engine
                    nop.bass_nofuse = True
                    nop.sync_info = mybir.SyncInfo(on_wait=[w], on_update=[])
                    nc.register_instruction(nop)
                    out.append(nop)
                inst.sync_info = mybir.SyncInfo(
                    on_wait=[waits[-1]], on_update=list(si.on_update or []))
            out.append(inst)
        insts[:] = out


def _build(causal: bool):
    nc = bass.Bass(num_devices=NCORE)

    xq = nc.declare_dram_parameter("xq", [D, S], F16, isOutput=False)
    xk = nc.declare_dram_parameter("xk", [D, S], F16, isOutput=False)
    xv = nc.declare_dram_parameter("xv", [D, S], F16, isOutput=False)
    wq = nc.declare_dram_parameter("wq", [D, DL], F16, isOutput=False)
    wk = nc.declare_dram_parameter("wk", [D, DL], F16, isOutput=False)
    wv = nc.declare_dram_parameter("wv", [D, DL], F16, isOutput=False)
    wo = nc.declare_dram_parameter("wo", [DL, D], F16, isOutput=False)
    out = nc.declare_dram_parameter("out", [2, 128, S], F16, isOutput=True)
    debug = os.environ.get("KERNEL_DEBUG", "0") == "1"
    if debug:
        d_qT = nc.declare_dram_parameter("d_qT", [2, 128, S], F16, isOutput=True)
        d_kT = nc.declare_dram_parameter("d_kT", [2, 128, S], F16, isOutput=True)
        d_Vp = nc.declare_dram_parameter("d_Vp", [128, NSK, 65 * HL], F16, isOutput=True)
        d_ctx = nc.declare_dram_parameter("d_ctx", [128, 2, S], F16, isOutput=True)

    with tile.TileContext(nc) as tc:
        with (
            tc.tile_pool(name="wpool", bufs=1) as wpool,
            tc.tile_pool(name="xpool", bufs=4) as xpool,
            tc.tile_pool(name="apool", bufs=1) as apool,
            tc.tile_pool(name="epool", bufs=4) as epool,
            tc.tile_pool(name="opool", bufs=2) as opool,
            tc.tile_pool(name="psS", bufs=2, space="PSUM") as psS,
            tc.tile_pool(name="psC", bufs=1, space="PSUM") as psC,
            tc.tile_pool(name="psP", bufs=1, space="PSUM") as psP,
            tc.tile_pool(name="dram", bufs=1, space="DRAM") as drp,
        ):
            # ---- resident weights / constants ----
            wq_sb = wpool.tile([128, KCH, DL], F16, tag="wq")
            wk_sb = wpool.tile([128, KCH, DL], F16, tag="wk")
            wv_sb = wpool.tile([128, KCH, DL], F16, tag="wv")
            wo_sb = wpool.tile([128, 2, D], F16, tag="wo")
            for wsb, wsrc in ((wq_sb, wq), (wk_sb, wk), (wv_sb, wv)):
                for c2 in range(2):  # split across queues
                    nc.scalar.dma_start(
                        wsb[:, 4 * c2:4 * (c2 + 1), :],
                        wsrc.rearrange("(c p) m -> p c m", p=128)[:, 4 * c2:4 * (c2 + 1), :])
            ones64f = wpool.tile([1, 64], F32, tag="ones64f")
            nc.vector.memset(ones64f[:], 1.0)
            ones64 = wpool.tile([1, 64], F32R, tag="ones64")
            nc.vector.tensor_copy(ones64[:], ones64f[:])
            # 0/1 strictly-lower-triangular (keep f >= p) mask, doubled along
            # the head axis; built once on gpsimd while its queue is empty
            mask01 = wpool.tile([128, 2, 128], F16, tag="mask01")
            nc.vector.memset(mask01[:], 1.0)
            nc.gpsimd.affine_select(
                mask01[:], mask01[:], pattern=[[0, 2], [1, 128]],
                compare_op=ALU.is_ge, fill=0.0, base=0,
                channel_multiplier=-1)

            # ---- persistent activations ----
            qT = [apool.tile([128, S], F16, tag=f"qT{hp}", name=f"qT{hp}")
                  for hp in range(2)]
            kT = [apool.tile([128, S], F16, tag=f"kT{hp}", name=f"kT{hp}")
                  for hp in range(2)]
            Vp = apool.tile([128, NSK, 65 * HL], F16, tag="Vp")
            nc.gpsimd.memset(
                Vp.rearrange("p i (h e) -> p i h e", e=65)[:, :, :, 64:65], 1.0)
            ctx_sb = apool.tile([128, 2, S], F16, tag="ctx")

            # ---- x resident: [128, KCH, S] per tensor ----
            # Full 4KB DRAM rows per descriptor (a 512-col piece would cut
            # per-descriptor bytes 4x and leave staging descriptor-rate
            # bound). Alternate the issuing engine: SP and ACT have separate
            # HWDGE queue sets.
            xk_sb = xpool.tile([128, KCH, S], F16, tag="xk", bufs=1)
            xv_sb = xpool.tile([128, KCH, S], F16, tag="xv", bufs=1)
            xq_sb = xpool.tile([128, KCH, S], F16, tag="xq", bufs=1)
            dma_eng = [nc.sync, nc.scalar]
            x_eng = [nc.sync, nc.scalar, nc.gpsimd]
            for (tl, src) in ((xk_sb, xk), (xv_sb, xv), (xq_sb, xq)):
                for kk in range(KCH):
                    x_eng[kk % 3].dma_start(
                        tl[:, kk, :], src[128 * kk:128 * (kk + 1), :])
            # wo is first needed by outproj(0) during attn(1); stage it last
            nc.scalar.dma_start(wo_sb[:], wo.rearrange("(c p) m -> p c m", p=128))
            xk_pc = [xk_sb[:, :, SQG * j:SQG * (j + 1)] for j in range(NSQG)]
            xv_pc = [xv_sb[:, :, SQG * j:SQG * (j + 1)] for j in range(NSQG)]
            xq_pc = [xq_sb[:, :, SQG * j:SQG * (j + 1)] for j in range(NSQG)]

            # ---------------- filler chain machinery ----------------
            # Each chain-unit is a list of thunks; consecutive thunks of the
            # open unit are popped between attention blocks. A unit owns one
            # psP tile for its whole life, so units must not interleave.
            fillers = deque()   # deque of lists (chain units); unit = deque of thunks

            def drain(n):
                """Emit up to n filler thunks (crossing unit boundaries)."""
                while n > 0 and fillers:
                    unit = fillers[0]
                    while n > 0 and unit:
                        unit.popleft()()
                        n -= 1
                    if not unit:
                        fillers.popleft()

            def drain_unit_boundary():
                """Finish the currently open chain unit (frees its psP tile)."""
                if fillers and fillers[0]:
                    unit = fillers.popleft()
                    while unit:
                        unit.popleft()()

            def drain_all():
                while fillers:
                    drain_unit_boundary()

            # ---------------- projection chain units ----------------
            def qk_proj_unit(j, xt, w_sb, dst):
                """One unit: both 128-row halves of q/k columns for group j."""
                unit = deque()
                state = {}

                def open_():
                    state["ps"] = psP.tile([128, 2, SQG], F32, tag="pj",
                                           name=f"pj_{id(state)}")
                for kk in range(KCH):
                    def mm(kk=kk):
                        if kk == 0:
                            open_()
                        ps = state["ps"]
                        for cc in range(2):
                            nc.tensor.matmul(
                                ps[:, cc, :],
                                lhsT=w_sb[:, kk, 128 * cc:128 * (cc + 1)],
                                rhs=xt[:, kk, :],
                                start=(kk == 0), stop=(kk == KCH - 1))
                    unit.append(mm)

                def close():
                    ps = state["ps"]
                    for cc in range(2):
                        nc.vector.tensor_copy(
                            dst[cc][:, SQG * j:SQG * (j + 1)], ps[:, cc, :])
                unit.append(close)
                return unit

            def v_proj_unit(j, half):
                """One unit: two sk-128-chunks of v for group j (natural)."""
                unit = deque()
                state = {}
                for kk in range(KCH):
                    def mm(kk=kk):
                        if kk == 0:
                            state["ps"] = psP.tile([128, 2, SQG], F32, tag="pj",
                                                   name=f"pv_{id(state)}")
                        ps = state["ps"]
                        for sc2 in range(2):
                            sc = 2 * half + sc2
                            nc.tensor.matmul(
                                ps[:, sc2, :DL],
                                lhsT=xv_pc[j][:, kk, 128 * sc:128 * (sc + 1)],
                                rhs=wv_sb[:, kk, :],
                                start=(kk == 0), stop=(kk == KCH - 1))
                    unit.append(mm)

                def close():
                    ps = state["ps"]
                    for sc2 in range(2):
                        sc = 2 * half + sc2
                        i = 4 * j + sc
                        vdst = Vp[:, i].rearrange("p (h e) -> p h e", e=65)
                        nc.vector.tensor_copy(
                            vdst[:, :, :64],
                            ps[:, sc2, :DL].rearrange("p (h e) -> p h e", e=64))
                unit.append(close)
                return unit

            def proj_units(j):
                """Chain units for group j in dependency-useful order."""
                return [
                    qk_proj_unit(j, xk_pc[j], wk_sb, kT),
                    v_proj_unit(j, 0),
                    v_proj_unit(j, 1),
                    qk_proj_unit(j, xq_pc[j], wq_sb, qT),
                ]

            # ---------------- out-projection + ReduceScatter ----------------
            def outproj_units(jg):
                """4 units x (2 oc chains of 2 MMs + copies); last unit also
                stages DRAM part + ReduceScatter + final out DMA."""
                par = {}

                def open_par():
                    par["sb"] = opool.tile([128, KCH, SQG], F16, tag="par",
                                           name=f"par{jg}")
                units = []
                for u in range(4):
                    unit = deque()
                    for oc2 in range(2):
                        oc = 2 * u + oc2
                        def mm(oc=oc, oc2=oc2, u=u):
                            if u == 0 and oc2 == 0:
                                open_par()
                            if oc2 == 0:
                                par["ps"] = psP.tile([128, 2, SQG], F32, tag="pj",
                                                     name=f"po{jg}_{u}")
                            ps = par["ps"]
                            for kc in range(2):
                                nc.tensor.matmul(
                                    ps[:, oc2, :],
                                    lhsT=wo_sb[:, kc, 128 * oc:128 * (oc + 1)],
                                    rhs=ctx_sb[:, kc, SQG * jg:SQG * (jg + 1)],
                                    start=(kc == 0), stop=(kc == 1))
                        unit.append(mm)

                    def close(u=u):
                        ps = par["ps"]
                        for oc2 in range(2):
                            nc.vector.tensor_copy(
                                par["sb"][:, 2 * u + oc2, :], ps[:, oc2, :])
                    unit.append(close)
                    units.append(unit)

                def ship():
                    part = drp.tile([KCH, 128, SQG], F16, name=f"part{jg}")
                    for oc in range(KCH):
                        # split across the SP and ACT HWDGE queue sets
                        dma_eng[oc % 2].dma_start(part[oc], par["sb"][:, oc, :])
                    rsout = drp.tile([2, 128, SQG], F16, name=f"rso{jg}")
                    nc.gpsimd.collective_compute(
                        "ReduceScatter", ALU.add, replica_groups=GROUPS,
                        ins=[part.opt()], outs=[rsout.opt()])
                    for h2 in range(2):
                        nc.sync.dma_start(
                            out[h2:h2 + 1, :, SQG * jg:SQG * (jg + 1)],
                            rsout[h2:h2 + 1])
                units[-1].append(ship)
                return units

            # ---------------- attention ----------------
            def attn_jg(jg):
                nsk = 4 * jg + 4 if causal else NSK
                for hp in range(2):
                    ctx_ps = [psC.tile([65, SQG], F32, tag=f"ctx{m}",
                                       name=f"ctx{jg}_{hp}_{m}")
                              for m in range(2)]
                    ets = {}

                    def pv(i):
                        et, c0 = ets.pop(i)
                        for m in range(2):
                            hl = 2 * hp + m
                            nc.tensor.matmul(
                                ctx_ps[m][:, c0:SQG],
                                lhsT=Vp[:, i, 65 * hl:65 * hl + 65],
                                rhs=et[:, m, c0:SQG],
                                start=(i == 0), stop=(i == nsk - 1))

                    for i in range(nsk):
                        col0 = 128 * max(0, i - 4 * jg) if causal else 0
                        sps = psS.tile([128, 2, SQG], F32, tag="sps",
                                       name=f"sps{jg}_{hp}_{i}")
                        for m in range(2):
                            nc.tensor.matmul(
                                sps[:, m, col0:SQG],
                                lhsT=kT[hp][64 * m:64 * m + 64,
                                            128 * i:128 * (i + 1)],
                                rhs=qT[hp][64 * m:64 * m + 64,
                                           SQG * jg + col0:SQG * (jg + 1)],
                                start=True, stop=True)
                        et = epool.tile([128, 2, SQG], F16, tag="exp",
                                        name=f"exp{jg}_{hp}_{i}")
                        nc.scalar.activation(
                            et[:, :, col0:SQG], sps[:, :, col0:SQG],
                            AF.Exp, scale=SCALE)
                        if causal and i >= 4 * jg:
                            # zero strictly-upper triangle of the diagonal
                            # 128x128 sub-block via the 0/1 tril mask. On
                            # DVE, NOT gpsimd: collective triggers block the
                            # gpsimd queue and would stall these (and with
                            # them the PV chain).
                            nc.vector.tensor_tensor(
                                et[:, :, col0:col0 + 128],
                                et[:, :, col0:col0 + 128],
                                mask01[:], ALU.mult)
                        ets[i] = (et, col0)
                        drain(2)
                        if i > 1:
                            pv(i - 2)
                    if nsk > 1:
                        pv(nsk - 2)
                    pv(nsk - 1)

                    # ---- softmax finalize ----
                    # Copy denominators AND raw ctx out of PSUM immediately:
                    # the ctx bank ring (bufs=1) gates the next (hp,jg)'s PV
                    # chain, so its last reader must come as early as
                    # possible. The recip/broadcast/scale then runs entirely
                    # from SBUF, overlapped with the next attention rows.
                    den = opool.tile([1, 2 * SQG], F32, tag="den",
                                     name=f"den{jg}_{hp}")
                    craw = opool.tile([128, SQG], F32, tag="craw",
                                      name=f"craw{jg}_{hp}")
                    for m in range(2):
                        nc.vector.tensor_copy(
                            den[:, SQG * m:SQG * (m + 1)], ctx_ps[m][64:65, :])
                        nc.vector.tensor_copy(
                            craw[64 * m:64 * m + 64, :], ctx_ps[m][0:64, :])
                    # DVE reciprocal is an 8-cycle/elem iterative divide
                    # (~3.3us on a [1,512] row). Spread the 1024 denominators
                    # over 16 partitions via a DRAM bounce, recip there at 64
                    # elem/lane, gather back; f32r out satisfies the
                    # f32r-matmul rounding rule without a cast copy.
                    den_sp = opool.tile([16, 64], F32, tag="densp",
                                        name=f"densp{jg}_{hp}")
                    nc.gpsimd.dma_start(den_sp[:], den[:])
                    rec_sp = opool.tile([16, 64], F32R, tag="recsp",
                                        name=f"recsp{jg}_{hp}")
                    with nc.allow_low_precision(reason="recip in f32r"):
                        nc.vector.reciprocal(rec_sp[:], den_sp[:])
                    rec = opool.tile([1, 2 * SQG], F32R, tag="rec",
                                     name=f"rec{jg}_{hp}")
                    nc.gpsimd.dma_start(rec[:], rec_sp[:])
                    drain_unit_boundary()
                    for m in range(2):
                        bc = psP.tile([128, 2, SQG], F32, tag="pj",
                                      name=f"bc{jg}_{hp}_{m}")
                        nc.tensor.matmul(bc[0:64, 0, :],
                                         lhsT=ones64[:],
                                         rhs=rec[:, SQG * m:SQG * (m + 1)],
                                         start=True, stop=True)
                        bc_sb = opool.tile([128, SQG], F32, tag="bcsb",
                                           name=f"bcsb{jg}_{hp}_{m}")
                        nc.vector.tensor_copy(
                            bc_sb[64 * m:64 * m + 64, :], bc[0:64, 0, :])
                        nc.vector.tensor_tensor(
                            ctx_sb[64 * m:64 * m + 64, hp,
                                   SQG * jg:SQG * (jg + 1)],
                            craw[64 * m:64 * m + 64, :],
                            bc_sb[64 * m:64 * m + 64, :], ALU.mult)

            # ---------------- schedule ----------------
            for u in proj_units(0):
                while u:
                    u.popleft()()
            for jg in range(NSQG):
                # everything queued before this point produces data attn(jg)
                # may read (proj of group jg) -- it must precede attn(jg) in
                # each engine's in-order stream or the PE queue deadlocks.
                drain_all()
                if jg >= 1:
                    fillers.extend(outproj_units(jg - 1))
                if jg + 1 < NSQG:
                    fillers.extend(proj_units(jg + 1))
                attn_jg(jg)
            drain_all()
            for u in outproj_units(NSQG - 1):
                while u:
                    u.popleft()()

            if debug:
                for hp in range(2):
                    nc.sync.dma_start(d_qT[hp], qT[hp][:])
                    nc.sync.dma_start(d_kT[hp], kT[hp][:])
                    nc.sync.dma_start(d_ctx[:, hp, :], ctx_sb[:, hp, :])
                nc.sync.dma_start(d_Vp[:], Vp[:])

    _split_multiwait(nc)
    return nc


def _mask_kind(mask: np.ndarray) -> bool:
    """True if causal (tril), False if all-ones; raises otherwise."""
    m = np.asarray(mask).reshape(S, S)
    if np.array_equal((m != 0).astype(np.int8), np.tril(np.ones((S, S), np.int8))):
        return True
    if np.all(m != 0):
        return False
    raise NotImplementedError("unsupported mask pattern")


def kernel(q, k, v, mask, w_q, b_q, w_k, b_k, w_v, b_v, w_o, b_o):
    global LAST_RESULT
    assert not np.any(b_q) and not np.any(b_k) and not np.any(b_v) \
        and not np.any(b_o), "nonzero biases not supported"
    causal = _mask_kind(mask)

    if causal not in _CACHE:
        _CACHE[causal] = _build(causal)
    nc = _CACHE[causal]

    f8 = np.float16
    q = np.asarray(q, np.float32) * XS
    k = np.asarray(k, np.float32) * XS
    v = np.asarray(v, np.float32) * XS
    # transposed per-batch activations, fp8 scaled by XS
    xqs = [np.ascontiguousarray(q[b].T).astype(f8) for b in range(B)]
    xks = [np.ascontiguousarray(k[b].T).astype(f8) for b in range(B)]
    xvs = [np.ascontiguousarray(v[b].T).astype(f8) for b in range(B)]
    # q/k/v weights fp8 scaled by WS; w_o folds away the v-path's XS*WS
    wqf = np.asarray(w_q, np.float32) * WS
    wkf = np.asarray(w_k, np.float32) * WS
    wvf = np.asarray(w_v, np.float32) * WS
    wof = np.asarray(w_o, np.float32) / PSC
    wqs = [np.ascontiguousarray(wqf[:, DL * g:DL * (g + 1)]).astype(f8) for g in range(4)]
    wks = [np.ascontiguousarray(wkf[:, DL * g:DL * (g + 1)]).astype(f8) for g in range(4)]
    wvs = [np.ascontiguousarray(wvf[:, DL * g:DL * (g + 1)]).astype(f8) for g in range(4)]
    wos = [np.ascontiguousarray(wof[DL * g:DL * (g + 1), :]).astype(np.float16) for g in range(4)]

    in_maps = []
    for c in range(NCORE):
        b, g = c // 4, c % 4
        in_maps.append({
            "xq": xqs[b], "xk": xks[b], "xv": xvs[b],
            "wq": wqs[g], "wk": wks[g], "wv": wvs[g], "wo": wos[g],
        })
    res = run_bass_kernel_spmd(nc, in_maps, core_ids=list(range(NCORE)))
    LAST_RESULT = res

    outf = np.empty((B, S, D), np.float32)
    for c in range(NCORE):
        b, g = c // 4, c % 4
        o = res.results[c]["out"].reshape(DL, S).astype(np.float32)
        outf[b, :, DL * g:DL * (g + 1)] = o.T
    return outf
